# revision 1
# baseline (speedup 1.0000x reference)
"""DeepSeek-MoE Trainium2 kernel (8 NeuronCores, expert-parallel).

Strategy
--------
* Routing (sigmoid + grouped top-k, DeepSeek noaux_tc) is replicated on every
  core in fp32 (top-k margins in this regime are ~2e-5, so bf16 routing would
  flip expert selections).
* Dispatch/combine are dense one-hot matmuls built on-device from the routing
  result: rank-within-expert comes from a cumsum over tokens realized as a
  matmul with triangular/ones masks, and the one-hot dispatch matrix
  D[t, c] = (rank[t, e_slot] == c) is built with tensor_scalar(is_equal).
* Expert parallelism: 4 experts per core (host bin-packing from the actual
  routing), per-slot capacities are compile-time 128-multiples.
* Expert weights are bf16 (halves HBM traffic; fp32 accumulation in PSUM).
* Shared experts are sharded over their intermediate dim (352 channels/core).
* Overlap schedule: w13 chunk DMAs are issued at program start (prefetch into
  a dedicated pool) so HBM never idles during routing; the shared-expert GEMM
  (PE) runs concurrently with the routing chain (DVE); one final combine pass
  accumulates all expert slots + shared in PSUM (no intermediate partial
  accumulate-DMA), and the cross-core ReduceScatter is split per h-half so
  RS(h0) overlaps combine(h1).
"""

import numpy as np
import ml_dtypes

T, H, E, K, I = 512, 2048, 32, 8, 1408
NG, TKG = 8, 4
RSF = 2.5
NCORES = 8
P = 128
ISH = 2 * I // NCORES  # 352: shared-expert intermediate slice per core
HT = H // P            # 16 h-tiles
TT = T // P            # 4 token tiles
IT = I // P            # 11 i-tiles
GS = E // NG           # 4 experts per group
BIG = 1.0e9

bf16 = ml_dtypes.bfloat16


# ----------------------------------------------------------------------------
# Host-side routing mirror (only used to pick expert->core assignment and
# compile-time slot capacities; the device re-computes routing exactly).
# ----------------------------------------------------------------------------
def _host_loads(x, gate_w, bias):
    logits = (x.astype(np.float32) @ gate_w.astype(np.float32)).astype(np.float32)
    scores = (1.0 / (1.0 + np.exp(-logits))).astype(np.float32)
    sb = scores + bias[None, :].astype(np.float32)
    g = sb.reshape(T, NG, GS)
    pair = [g[..., i] + g[..., j] for i in range(GS) for j in range(i + 1, GS)]
    grp = np.max(np.stack(pair, -1), -1)
    gmask = np.zeros((T, NG), np.float32)
    gw = grp.copy()
    for _ in range(TKG):
        mx = gw.max(-1, keepdims=True)
        eq = (gw == mx).astype(np.float32)
        gmask += eq
        gw -= eq * BIG
    emask = np.repeat(gmask, GS, axis=1)
    m = sb + (emask * BIG - BIG)
    kmask = np.zeros((T, E), np.float32)
    for _ in range(K):
        mx = m.max(-1, keepdims=True)
        eq = (m == mx).astype(np.float32)
        kmask += eq
        m -= eq * BIG
    return kmask.sum(0)


def _plan_slots(loads, margin=2):
    caps = (np.ceil((loads + margin) / P).astype(int) * P).clip(P, None)
    order = np.argsort(-(caps * 1000 + loads))
    groups = [[] for _ in range(NCORES)]
    gsum = [0] * NCORES
    for e in order:
        cand = [i for i in sorted(range(NCORES), key=lambda i: (gsum[i], len(groups[i])))
                if len(groups[i]) < 4]
        i = cand[0]
        groups[i].append(int(e))
        gsum[i] += caps[e]
    for i in range(NCORES):
        groups[i].sort(key=lambda e: -caps[e])
    slot_caps = [int(max(caps[groups[i][j]] for i in range(NCORES))) for j in range(4)]
    return groups, slot_caps


# ----------------------------------------------------------------------------
# Device program
# ----------------------------------------------------------------------------
def _build_nc(slot_caps, single_core=False):
    import concourse.mybir as mybir
    import concourse.tile as tile
    from concourse import bacc
    from contextlib import ExitStack

    f32 = mybir.dt.float32
    b16 = mybir.dt.bfloat16
    Alu = mybir.AluOpType
    Act = mybir.ActivationFunctionType
    Ax = mybir.AxisListType

    cts = [c // P for c in slot_caps]            # ctiles per slot
    offs = np.cumsum([0] + slot_caps).tolist()   # D column offsets
    DCOLS = offs[-1]
    NCT = sum(cts)                               # total ctiles on this core
    cbase = np.cumsum([0] + cts).tolist()        # global ctile index base per slot
    CAPMAX = max(slot_caps)

    nc = bacc.Bacc("TRN2", target_bir_lowering=False, debug=False,
                   num_devices=1 if single_core else NCORES)

    # ---- I/O ----
    x_d = nc.dram_tensor("x", [T, H], f32, kind="ExternalInput")
    gw_d = nc.dram_tensor("gate_w", [H, E], f32, kind="ExternalInput")
    bias_d = nc.dram_tensor("bias_b", [P, E], f32, kind="ExternalInput")
    w13_d = nc.dram_tensor("w13s", [4, H, 2 * I], b16, kind="ExternalInput")
    w2_d = nc.dram_tensor("w2s", [4, I, H], b16, kind="ExternalInput")
    wgu_d = nc.dram_tensor("wgu_sh", [H, 2 * ISH], b16, kind="ExternalInput")
    wdn_d = nc.dram_tensor("wdn_sh", [ISH, H], b16, kind="ExternalInput")
    sel_d = nc.dram_tensor("sel", [E, 4], f32, kind="ExternalInput")
    iota_d = nc.dram_tensor("iota_r", [P, CAPMAX], f32, kind="ExternalInput")
    triu_d = nc.dram_tensor("triu_b", [P, P], b16, kind="ExternalInput")
    ones_d = nc.dram_tensor("ones_b", [P, P], b16, kind="ExternalInput")
    id32_d = nc.dram_tensor("id_f32", [P, P], f32, kind="ExternalInput")
    id16_d = nc.dram_tensor("id_b16", [P, P], b16, kind="ExternalInput")
    out_d = nc.dram_tensor("out_slice",
                           [T, H] if single_core else [T // NCORES, H], f32,
                           kind="ExternalOutput")

    partial_d = [nc.dram_tensor(f"partial{i}", [T, H // 2], f32,
                                kind="Internal") for i in range(2)]
    rs_d = [nc.dram_tensor(f"rs_out{i}", [T // NCORES, H // 2], f32,
                           kind="Internal") for i in range(2)]

    def cp(i, out, in_):
        # alternate psum/sbuf copies between DVE and ACT to balance engines
        if i % 2 == 0:
            nc.vector.tensor_copy(out=out, in_=in_)
        else:
            nc.scalar.copy(out, in_)

    xr = x_d.ap().rearrange("(tt p) h -> p tt h", p=P)
    gwr = gw_d.ap().rearrange("(ko p) e -> p ko e", p=P)
    w13r = w13_d.ap().rearrange("j (ko p) f -> j p ko f", p=P)
    w2r = w2_d.ap().rearrange("j (ko p) h -> j p ko h", p=P)
    wgur = wgu_d.ap().rearrange("(ko p) f -> p ko f", p=P)

    with tile.TileContext(nc) as tc, ExitStack() as ctx:
        pc = ctx.enter_context(tc.tile_pool(name="persist", bufs=1))
        xp = ctx.enter_context(tc.tile_pool(name="xstream", bufs=2))
        wgp = ctx.enter_context(tc.tile_pool(name="w13stream", bufs=4))
        wp = ctx.enter_context(tc.tile_pool(name="wstream", bufs=3))
        ap_ = ctx.enter_context(tc.tile_pool(name="acts", bufs=1))
        tp_ = ctx.enter_context(tc.tile_pool(name="tmps", bufs=2))
        sp = ctx.enter_context(tc.tile_pool(name="smalls", bufs=1))
        psA = ctx.enter_context(tc.tile_pool(name="psumA", bufs=4, space="PSUM"))
        psB = ctx.enter_context(tc.tile_pool(name="psumB", bufs=1, space="PSUM"))

        def mmw(k, name):
            # two rotating 2-bank wide accumulators
            return psB.tile([P, 1024], f32, tag=f"mmw{k % 2}", name=name)

        # ---- w13 prefetch: issue the first chunks' DMAs at program start ----
        # chunk plan (consumption order): per slot j, per f-chunk fci, per
        # 4-ko group kg -> [P, 4, <=1024] bf16 (1MB). 12 chunks/slot, 48 total.
        FCH = []
        fo = 0
        while fo < I:
            FCH.append((fo, min(512, I - fo)))
            fo += 512
        KG = 4
        chunk_descs = []
        for j in range(4):
            for fci, (fo, fw) in enumerate(FCH):
                for kg in range(HT // KG):
                    chunk_descs.append((j, fci, kg, 1024 * fci, fw))
        # stage depth 6: 4 dedicated buffers (prefetched from t=0) plus 2
        # buffers borrowed from the x-stream pool once phase 1 releases them
        NSTAGE = 6
        wg_tiles = {}

        def load_chunk(i):
            j, fci, kg, co, fw = chunk_descs[i]
            if i % NSTAGE < 4:
                wg = wgp.tile([P, KG, 1024], b16, tag="wg", name="wg")
            else:
                wg = xp.tile([P, KG, 1024], b16, tag="xf", name="wg")
            nc.scalar.dma_start(
                wg[:, :, :2 * fw],
                w13r[j][:, kg * KG:(kg + 1) * KG, co:co + 2 * fw])
            wg_tiles[i] = wg

        for i in range(4):
            load_chunk(i)

        # ---- small constant loads ----
        gw_sb = pc.tile([P, HT, E], f32, tag="gw")
        nc.sync.dma_start(gw_sb[:], gwr)
        bias_sb = pc.tile([P, E], f32, tag="bias")
        nc.sync.dma_start(bias_sb[:], bias_d.ap())
        sel_sb = pc.tile([E, 4], f32, tag="sel")
        nc.sync.dma_start(sel_sb[:], sel_d.ap())
        iota_sb = pc.tile([P, CAPMAX], f32, tag="iota")
        nc.sync.dma_start(iota_sb[:], iota_d.ap())
        triu_sb = pc.tile([P, P], b16, tag="triu")
        nc.sync.dma_start(triu_sb[:], triu_d.ap())
        ones_sb = pc.tile([P, P], b16, tag="ones")
        nc.sync.dma_start(ones_sb[:], ones_d.ap())
        id32_sb = pc.tile([P, P], f32, tag="id32")
        nc.sync.dma_start(id32_sb[:], id32_d.ap())
        id16_sb = pc.tile([P, P], b16, tag="id16")
        nc.sync.dma_start(id16_sb[:], id16_d.ap())

        # ---- stream x in 256-col chunks: cast to bf16, build x^T (PE),
        # logits accumulated in a persistent PSUM tile across all 16 ko.
        # Logits matmuls for chunk hc-1 are issued after the transposes of
        # chunk hc, so the PE never stalls on the psum->sbuf copies. ----
        x_bf = pc.tile([P, TT, H], b16, tag="xb")
        xT_bf = pc.tile([P, HT, T], b16, tag="xTb")
        lg_sb = pc.tile([P, TT, E], f32, tag="lg")
        xtf_hist = []

        def logits_mms(s, xtf1):
            # each (step, tt) is its own tiny psum group (psum accumulation
            # groups cannot interleave within a bank); DVE accumulates into
            # lg_sb off the PE critical path
            for tt in range(TT):
                plc = psA.tile([P, E], f32, tag="sm", name="plc")
                for hl in range(2):
                    nc.tensor.matmul(plc[:], xtf1[:, hl, tt * P:(tt + 1) * P],
                                     gw_sb[:, 2 * s + hl, :],
                                     start=(hl == 0), stop=(hl == 1))
                if s == 0:
                    nc.vector.tensor_copy(out=lg_sb[:, tt, :], in_=plc[:])
                else:
                    nc.vector.tensor_tensor(lg_sb[:, tt, :], lg_sb[:, tt, :],
                                            plc[:], Alu.add)

        # shared-expert gate/up runs in two token-tile halves (PSUM limit):
        # th=0 is woven into the x-stream (its ko accumulation follows x^T
        # availability), th=1 re-streams wgu and overlaps the routing chain.
        act_sh = pc.tile([P, TT, ISH], b16, tag="actsh")
        psh_cur = [None, None]

        def shared_mms(s, th, pshs):
            wguc = wp.tile([P, 2, 2 * ISH], b16, tag="wguc", bufs=2)
            nc.sync.dma_start(wguc[:], wgur[:, 2 * s:2 * s + 2, :])
            for kl in range(2):
                ko = 2 * s + kl
                for ttl in range(2):
                    tt = th * 2 + ttl
                    for q0 in range(0, 2 * ISH, 512):
                        qw = min(512, 2 * ISH - q0)
                        nc.tensor.matmul(
                            pshs[ttl][:, q0:q0 + qw],
                            xT_bf[:, ko, tt * P:(tt + 1) * P],
                            wguc[:, kl, q0:q0 + qw],
                            start=(ko == 0), stop=(ko == HT - 1))

        def shared_act(th, pshs):
            # GPSIMD cannot touch PSUM: silu + up-copy run on Act (psum
            # readers), the multiply runs on Pool with SBUF operands so the
            # DVE queue stays free for the routing chain.
            for ttl in range(2):
                tt = th * 2 + ttl
                nc.scalar.activation(act_sh[:, tt, :], pshs[ttl][:, :ISH],
                                     Act.Silu)
                up_sb = tp_.tile([P, ISH], b16, tag="upsh", bufs=1)
                nc.scalar.copy(up_sb[:], pshs[ttl][:, ISH:2 * ISH])
                nc.gpsimd.tensor_tensor(act_sh[:, tt, :], act_sh[:, tt, :],
                                        up_sb[:], Alu.mult)

        psh0 = [mmw(0, "psh0"), mmw(1, "psh1")]
        xf_tiles = {}
        for s in range(8):
            hc, hlp = s // 2, s % 2
            if hlp == 0:
                xf = xp.tile([P, TT, 512], f32, tag="xf", name="xf")
                nc.sync.dma_start(xf[:], xr[:, :, hc * 512:(hc + 1) * 512])
                # bf16 cast on the otherwise-idle Pool engine
                nc.gpsimd.tensor_copy(
                    out=x_bf[:, :, hc * 512:(hc + 1) * 512], in_=xf[:])
                xf_tiles[hc] = xf
            xf = xf_tiles[hc]
            xtf1 = xp.tile([P, 2, T], f32, tag="xtf1")  # fp32 x^T h-slices
            for tt in range(TT):
                # two h-slices transposed into one psum tile -> batched copies
                pt = psA.tile([P, 256], f32, tag="sm", name="pt_x")
                for hl in range(2):
                    nc.tensor.transpose(
                        pt[:, hl * P:(hl + 1) * P],
                        xf[:, tt, hlp * 256 + hl * P:hlp * 256 + (hl + 1) * P],
                        id32_sb[:])
                ptv = pt.rearrange("p (k c) -> p k c", k=2)
                nc.vector.tensor_copy(out=xtf1[:, :, tt * P:(tt + 1) * P],
                                      in_=ptv)
                nc.scalar.copy(xT_bf[:, 2 * s:2 * s + 2,
                                     tt * P:(tt + 1) * P], ptv)
            xtf_hist.append(xtf1)
            if s > 0:
                logits_mms(s - 1, xtf_hist[s - 1])
                shared_mms(s - 1, 0, psh0)
        logits_mms(7, xtf_hist[7])
        shared_mms(7, 0, psh0)
        shared_act(0, psh0)
        # phase 1 done: the two xf buffers become w13 stage slots 4/5
        load_chunk(4)
        load_chunk(5)
        # th=1 re-streams wgu; its PE matmuls overlap the routing chain below
        psh1 = [mmw(2, "psh2"), mmw(3, "psh3")]
        for s in range(8):
            shared_mms(s, 1, psh1)
        shared_act(1, psh1)
        # transpose act_sh -> [i_s, t]
        actShT = pc.tile([P, 3, T], b16, tag="actShT")
        for tt in range(TT):
            for io in range(3):
                iw = min(P, ISH - io * P)
                pt3 = psA.tile([P, P], b16, tag="sm", name="pt3")
                nc.tensor.transpose(pt3[:iw, :],
                                    act_sh[:, tt, io * P:io * P + iw],
                                    id16_sb[:])
                cp(io + tt, actShT[:iw, io, tt * P:(tt + 1) * P], pt3[:iw, :])

        # ---- routing (fp32, on [P, TT, NG, GS] layouts) ----
        scores = pc.tile([P, TT, NG, GS], f32, tag="scores")
        nc.scalar.activation(scores.rearrange("p t g s -> p t (g s)"),
                             lg_sb[:], Act.Sigmoid)
        sbb = pc.tile([P, TT, NG, GS], f32, tag="sbb")
        nc.vector.tensor_tensor(
            sbb[:], scores[:],
            bias_sb.rearrange("p (g s) -> p g s", g=NG)[:, None, :, :]
            .to_broadcast([P, TT, NG, GS]), Alu.add)

        grp = sp.tile([P, TT, NG], f32, tag="grp")
        tmpg = sp.tile([P, TT, NG], f32, tag="tmpg")
        pw = tmpg
        first = True
        for i in range(GS):
            for j in range(i + 1, GS):
                dst = grp if first else pw
                nc.vector.tensor_tensor(dst[:], sbb[:, :, :, i], sbb[:, :, :, j],
                                        Alu.add)
                if not first:
                    nc.vector.tensor_tensor(grp[:], grp[:], pw[:], Alu.max)
                first = False

        gmask = sp.tile([P, TT, NG], f32, tag="gmask")
        # top-4-of-8 groups via hardware max8: sort all 8, pad the bottom 4
        # with an absent sentinel so match_replace marks only the top 4
        mx8g = sp.tile([P, TT, 8], f32, tag="mx8")
        for tt in range(TT):
            nc.vector.max(mx8g[:, tt, :], grp[:, tt, :])
            nc.vector.memset(mx8g[:, tt, TKG:], 2.0e9)
            nc.vector.match_replace(tmpg[:, tt, :], mx8g[:, tt, :],
                                    grp[:, tt, :], -3.0e9)
            nc.vector.tensor_scalar(gmask[:, tt, :], tmpg[:, tt, :], -2.0e9,
                                    None, Alu.is_lt)

        kmask = pc.tile([P, TT, E], f32, tag="kmask")
        tmpk = sp.tile([P, TT, E], f32, tag="tmpk")
        tmpk_r = tmpk.rearrange("p t (g s) -> p t g s", g=NG)
        nc.vector.tensor_scalar(tmpk_r, gmask[:, :, :, None]
                                .to_broadcast([P, TT, NG, GS]),
                                BIG, -BIG, Alu.mult, Alu.add)
        nc.vector.tensor_tensor(sbb[:], sbb[:], tmpk_r, Alu.add)
        m_f = sbb.rearrange("p t g s -> p t (g s)")
        # top-8-of-32 per token in 3 DVE ops per token tile: the hardware
        # max8 instruction + match_replace (selected entries -> -3e9, far
        # below the -1e9 group-masked floor), then one threshold compare
        mx8 = sp.tile([P, TT, 8], f32, tag="mx8")
        for tt in range(TT):
            nc.vector.max(mx8[:, tt, :], m_f[:, tt, :])
            nc.vector.match_replace(tmpk[:, tt, :], mx8[:, tt, :],
                                    m_f[:, tt, :], -3.0e9)
            nc.vector.tensor_scalar(kmask[:, tt, :], tmpk[:, tt, :], -2.0e9,
                                    None, Alu.is_lt)

        wsel = sp.tile([P, TT, E], f32, tag="wsel")
        nc.vector.tensor_tensor(wsel[:], kmask[:],
                                scores.rearrange("p t g s -> p t (g s)"),
                                Alu.mult)
        denom = sp.tile([P, TT], f32, tag="denom")
        nc.vector.reduce_sum(denom[:], wsel[:], axis=Ax.X)
        winv = sp.tile([P, TT], f32, tag="winv")
        nc.vector.reciprocal(winv[:], denom[:])
        nc.vector.tensor_scalar(winv[:], winv[:], RSF, None, Alu.mult)
        W_t = pc.tile([P, TT, E], f32, tag="Wt")
        nc.vector.tensor_tensor(W_t[:], wsel[:],
                                winv[:, :, None].to_broadcast([P, TT, E]),
                                Alu.mult)


        # ---- rank-within-expert cumsum + slot base (PE triangular matmuls) ----
        count_bf = sp.tile([P, TT, E], b16, tag="countb")
        nc.scalar.copy(count_bf[:], kmask[:])
        baseA = pc.tile([P, TT, E], f32, tag="baseA")
        namask = sp.tile([P, TT, E], f32, tag="namask")
        nc.vector.tensor_scalar(namask[:], kmask[:], -1.0e6, 1.0e6,
                                Alu.mult, Alu.add)
        for mt in range(TT):
            pb = psA.tile([P, E], f32, tag="sm", name="pb")
            for kk in range(mt + 1):
                lhs = ones_sb if kk < mt else triu_sb
                nc.tensor.matmul(pb[:], lhs[:], count_bf[:, kk, :],
                                 start=(kk == 0), stop=(kk == mt))
            nc.vector.tensor_tensor(baseA[:, mt, :], pb[:], namask[:, mt, :],
                                    Alu.add)

        # transpose baseA, W -> [E, t]; select this core's 4 experts via sel
        baT = pc.tile([E, TT, P], f32, tag="baT")
        wT = pc.tile([E, TT, P], f32, tag="wT")
        for tt in range(TT):
            pt1 = psA.tile([E, P], f32, tag="sm", name="pt1")
            nc.tensor.transpose(pt1[:], baseA[:, tt, :], id32_sb[:])
            nc.vector.tensor_copy(out=baT[:, tt, :], in_=pt1[:])
            pt2 = psA.tile([E, P], f32, tag="sm", name="pt2")
            nc.tensor.transpose(pt2[:], W_t[:, tt, :], id32_sb[:])
            nc.scalar.copy(wT[:, tt, :], pt2[:])
        bsel = pc.tile([P, TT, 4], f32, tag="bsel")
        wsel4 = pc.tile([P, TT, 4], f32, tag="wsel4")
        for tt in range(TT):
            pb4 = psA.tile([P, 4], f32, tag="sm", name="pb4")
            nc.tensor.matmul(pb4[:], baT[:, tt, :], sel_sb[:], start=True,
                             stop=True)
            nc.vector.tensor_copy(out=bsel[:, tt, :], in_=pb4[:])
            pw4 = psA.tile([P, 4], f32, tag="sm", name="pw4")
            nc.tensor.matmul(pw4[:], wT[:, tt, :], sel_sb[:], start=True,
                             stop=True)
            nc.scalar.copy(wsel4[:, tt, :], pw4[:])

        # dispatch one-hot D (bf16); combine weights Wc built blockwise -> WcT
        D_sb = pc.tile([P, TT, DCOLS], b16, tag="D")
        WcT = pc.tile([P, NCT, T], b16, tag="WcT")
        for tt in range(TT):
            for j in range(4):
                cap = slot_caps[j]
                nc.vector.tensor_scalar(D_sb[:, tt, offs[j]:offs[j] + cap],
                                        iota_sb[:, :cap], bsel[:, tt, j:j + 1],
                                        None, Alu.is_equal)
                wcs = sp.tile([P, 256], f32, tag="wcs")
                nc.vector.tensor_scalar(wcs[:, :cap], iota_sb[:, :cap],
                                        bsel[:, tt, j:j + 1],
                                        wsel4[:, tt, j:j + 1],
                                        Alu.is_equal, Alu.mult)
                for cl in range(cts[j]):
                    ptw = psA.tile([P, P], f32, tag="sm", name="ptw")
                    nc.tensor.transpose(ptw[:], wcs[:, cl * P:(cl + 1) * P],
                                        id32_sb[:])
                    cp(cl + tt, WcT[:, cbase[j] + cl, tt * P:(tt + 1) * P],
                       ptw[:])

        # ---- dispatch matmul: xeT[h, c] = sum_t x[t,h] D[t,c] ----
        xeT = pc.tile([P, HT, DCOLS], b16, tag="xeT")
        NDW = (DCOLS + 1023) // 1024
        pctr = 0
        for ko in range(HT):
            for dch in range(NDW):
                cw = min(1024, DCOLS - dch * 1024)
                px = mmw(pctr, "px")
                pctr += 1
                for tt in range(TT):
                    for q0 in range(0, cw, 512):
                        qw = min(512, cw - q0)
                        nc.tensor.matmul(
                            px[:, q0:q0 + qw],
                            x_bf[:, tt, ko * P:(ko + 1) * P],
                            D_sb[:, tt, dch * 1024 + q0:dch * 1024 + q0 + qw],
                            start=(tt == 0), stop=(tt == TT - 1))
                cp(ko + dch, xeT[:, ko, dch * 1024:dch * 1024 + cw],
                   px[:, :cw])

        # ---- routed experts ----
        # w13s is host-packed: per 512-f-chunk, gate|up columns adjacent
        ye_tiles = []
        chunk_i = 0
        for j in range(4):
            ct = cts[j]
            act = ap_.tile([P, 2, I], b16, tag="act", name="act")
            for fci, (fo, fw) in enumerate(FCH):
                pgus = [mmw(pctr + ci, f"pgu{ci}") for ci in range(ct)]
                pctr += ct
                for kg in range(HT // KG):
                    if chunk_i not in wg_tiles:
                        load_chunk(chunk_i)
                    wg = wg_tiles.pop(chunk_i)
                    nxt = chunk_i + NSTAGE
                    chunk_i += 1
                    if nxt < len(chunk_descs) and nxt not in wg_tiles:
                        load_chunk(nxt)
                    for kl in range(KG):
                        ko = kg * KG + kl
                        for ci in range(ct):
                            lhs = xeT[:, ko,
                                      offs[j] + ci * P: offs[j] + (ci + 1) * P]
                            for q0 in range(0, 2 * fw, 512):
                                qw = min(512, 2 * fw - q0)
                                nc.tensor.matmul(pgus[ci][:, q0:q0 + qw], lhs,
                                                 wg[:, kl, q0:q0 + qw],
                                                 start=(ko == 0),
                                                 stop=(ko == HT - 1))
                for ci in range(ct):
                    tmpa = tp_.tile([P, 1024], b16, tag="stg")
                    nc.scalar.activation(tmpa[:, :fw], pgus[ci][:, :fw], Act.Silu)
                    nc.vector.tensor_tensor(act[:, ci, fo:fo + fw],
                                            tmpa[:, :fw], pgus[ci][:, fw:2 * fw],
                                            Alu.mult)
            # transpose act -> actT [i, c]
            actT = ap_.tile([P, IT, 256], b16, tag="actT", name="actT")
            for ci in range(ct):
                for io in range(IT):
                    pt4 = psA.tile([P, P], b16, tag="sm", name="pt4")
                    nc.tensor.transpose(pt4[:], act[:, ci, io * P:(io + 1) * P],
                                        id16_sb[:])
                    cp(io, actT[:, io, ci * P:(ci + 1) * P], pt4[:])
            # down-proj ye[c, h] in two h-halves, w2 streamed once per half
            yes = [pc.tile([P, H], b16, tag=f"ye{cbase[j] + ci}",
                           name=f"ye{cbase[j] + ci}") for ci in range(ct)]
            ye_tiles.extend(yes)
            KOG = [(0, 3), (3, 3), (6, 3), (9, 2)]

            def down_half(j, ct, actT, yes, hh):
                nonlocal pctr
                pys = [mmw(pctr + ci, f"py{ci}") for ci in range(ct)]
                pctr += ct
                for (ko0, kn) in KOG:
                    w2c = wp.tile([P, 3, 1024], b16, tag="w2s", name="w2c")
                    nc.sync.dma_start(
                        w2c[:, :kn, :],
                        w2r[j][:, ko0:ko0 + kn,
                               hh * 1024:(hh + 1) * 1024])
                    for kl in range(kn):
                        ko = ko0 + kl
                        for ci in range(ct):
                            for q0 in (0, 512):
                                nc.tensor.matmul(
                                    pys[ci][:, q0:q0 + 512],
                                    actT[:, ko, ci * P:(ci + 1) * P],
                                    w2c[:, kl, q0:q0 + 512],
                                    start=(ko == 0), stop=(ko == IT - 1))
                for ci in range(ct):
                    cp(ci + hh, yes[ci][:, hh * 1024:(hh + 1) * 1024],
                       pys[ci][:])

            def combine(hh):
                # all slots + shared accumulated in PSUM, one partial write
                # per (hh, tt, half); RS per h-half so RS(h0) overlaps the
                # remaining h1 compute
                nonlocal pctr
                wdnc = wp.tile([P, 3, 1024], b16, tag="w2s", name="wdnc")
                for io in range(3):
                    iw = min(P, ISH - io * P)
                    nc.sync.dma_start(
                        wdnc[:iw, io, :],
                        wdn_d.ap()[io * P:io * P + iw,
                                   hh * 1024:(hh + 1) * 1024])
                for tt in range(TT):
                    po = mmw(pctr, f"po{pctr % 2}")
                    pctr += 1
                    for q, cb in enumerate(range(NCT)):
                        for q0 in (0, 512):
                            nc.tensor.matmul(
                                po[:, q0:q0 + 512],
                                WcT[:, cb, tt * P:(tt + 1) * P],
                                ye_tiles[cb][:, hh * 1024 + q0:
                                             hh * 1024 + q0 + 512],
                                start=(q == 0), stop=False)
                    for io in range(3):
                        iw = min(P, ISH - io * P)
                        for q0 in (0, 512):
                            nc.tensor.matmul(
                                po[:, q0:q0 + 512],
                                actShT[:iw, io, tt * P:(tt + 1) * P],
                                wdnc[:iw, io, q0:q0 + 512],
                                start=False, stop=(io == 2))
                    for sh in range(2):
                        stg = tp_.tile([P, 512], f32, tag="stg")
                        cp(tt + sh, stg[:], po[:, sh * 512:(sh + 1) * 512])
                        if single_core:
                            dst = out_d.ap()[tt * P:(tt + 1) * P,
                                             hh * 1024 + sh * 512:
                                             hh * 1024 + (sh + 1) * 512]
                        else:
                            dst = partial_d[hh].ap()[
                                tt * P:(tt + 1) * P,
                                sh * 512:(sh + 1) * 512]
                        nc.gpsimd.dma_start(dst, stg[:])
                oslc = out_d.ap()[:, hh * 1024:(hh + 1) * 1024]
                if not single_core:
                    nc.gpsimd.collective_compute(
                        "ReduceScatter", Alu.add,
                        replica_groups=[list(range(NCORES))],
                        ins=[partial_d[hh].ap().opt()],
                        outs=[rs_d[hh].ap().opt()],
                    )
                    nc.sync.dma_start(oslc, rs_d[hh].ap())

            if j < 3:
                down_half(j, ct, actT, yes, 0)
                down_half(j, ct, actT, yes, 1)
            else:
                # overlap: combine(h0) runs while j3's h1 down-proj streams
                down_half(j, ct, actT, yes, 0)
                combine(0)
                down_half(j, ct, actT, yes, 1)
                combine(1)

    nc.compile()
    return nc


_NC_CACHE = {}


def _pack_inputs(x, gate_w, bias, w13, w2, sgu, sdn, groups, slot_caps):
    """Per-core in_maps. w13 is packed so each 512-wide f-chunk has its gate
    and up columns adjacent: [g0|u0|g1|u1|g2|u2] with chunk widths 512/512/384."""
    CAPMAX = max(slot_caps)
    iota = np.tile(np.arange(CAPMAX, dtype=np.float32), (P, 1))
    triu = np.triu(np.ones((P, P), np.float32), 1).astype(bf16)
    ones = np.ones((P, P), bf16)
    id32 = np.eye(P, dtype=np.float32)
    id16 = np.eye(P, dtype=np.float32).astype(bf16)
    bias_b = np.tile(bias[None, :], (P, 1)).astype(np.float32)

    def pack_w13(w):   # w: [H, 2I] fp32 -> packed bf16
        cols = []
        fo = 0
        while fo < I:
            fw = min(512, I - fo)
            cols.append(w[:, fo:fo + fw])
            cols.append(w[:, I + fo:I + fo + fw])
            fo += fw
        return np.ascontiguousarray(np.concatenate(cols, axis=1)).astype(bf16)

    in_maps = []
    for core in range(NCORES):
        sel = np.zeros((E, 4), np.float32)
        for j, e in enumerate(groups[core]):
            sel[e, j] = 1.0
        gsl = slice(core * ISH, (core + 1) * ISH)
        wgu_sh = np.concatenate(
            [sgu[:, gsl], sgu[:, 2 * I + core * ISH: 2 * I + (core + 1) * ISH]],
            axis=1).astype(bf16)
        in_maps.append({
            "x": x, "gate_w": gate_w, "bias_b": bias_b,
            "w13s": np.stack([pack_w13(w13[e]) for e in groups[core]]),
            "w2s": np.ascontiguousarray(w2[groups[core]]).astype(bf16),
            "wgu_sh": np.ascontiguousarray(wgu_sh),
            "wdn_sh": np.ascontiguousarray(
                sdn[core * ISH:(core + 1) * ISH, :]).astype(bf16),
            "sel": sel, "iota_r": iota, "triu_b": triu, "ones_b": ones,
            "id_f32": id32, "id_b16": id16,
        })
    return in_maps


def kernel(hidden_states, residual, gate_w, bias, w13, w2, shared_gate_up,
           shared_down):
    from concourse.bass_utils import run_bass_kernel_spmd

    x = np.ascontiguousarray(np.asarray(hidden_states, np.float32))
    gate_w = np.ascontiguousarray(np.asarray(gate_w, np.float32))
    bias = np.asarray(bias, np.float32)
    w13 = np.asarray(w13, np.float32)
    w2 = np.asarray(w2, np.float32)
    sgu = np.asarray(shared_gate_up, np.float32)
    sdn = np.asarray(shared_down, np.float32)

    loads = _host_loads(x, gate_w, bias)
    groups, slot_caps = _plan_slots(loads)

    key = tuple(slot_caps)
    if key not in _NC_CACHE:
        _NC_CACHE[key] = _build_nc(slot_caps)
    nc = _NC_CACHE[key]

    in_maps = _pack_inputs(x, gate_w, bias, w13, w2, sgu, sdn, groups,
                           slot_caps)
    res = run_bass_kernel_spmd(nc, in_maps, core_ids=list(range(NCORES)))
    out = np.concatenate([res.results[c]["out_slice"] for c in range(NCORES)],
                         axis=0)
    return out.astype(np.float32)



# revision 11
# speedup vs baseline: 1.3274x; 1.3274x over previous
"""DeepSeek-MoE Trainium2 kernel (8 NeuronCores, expert-parallel).

Strategy
--------
* Routing (sigmoid + grouped top-k, DeepSeek noaux_tc) is computed on the
  HOST in fp32 (exact mirror of the reference ops). The device consumes the
  routing results as dense inputs: a one-hot dispatch matrix D[t, c] and the
  transposed combine-weight matrix WcT[c, t] (weights * RSF, renormalized).
  This removes the fp32 x load, the logits GEMM and the whole on-device
  routing chain from the kernel.
* Expert parallelism: 4 experts per core, assigned rank-strided by load so
  the per-slot max capacity across cores (the SPMD program is shared) stays
  tight. Capacities are exact loads rounded to 8 (no 128 padding).
* Expert GEMMs are WEIGHT-STATIONARY: stationary = weight tile [k, 128],
  moving = activations [k, cap_e]. PE matmul cost is out_cols x k_tiles
  regardless of partition fill, so exact (non-128-padded) capacities cut
  ~33% of the expert-GEMM PE time vs. token-stationary tiles.
* Shared experts are sharded over the intermediate dim (352 ch/core) and run
  f-major (stationary = wgu tile) so wgu streams exactly once and the
  activations land directly in [i_s, t] layout for the combine.
* Everything on device is bf16 except PSUM accumulation; output partials are
  bf16 (host upcasts to fp32 after the ReduceScatter).
* Schedule: one SP-queue DMA stream (x, D, WcT, wgu, wdn, then w13/w2 chunks
  in consumption order with lookahead); PE head = x-transposes + dispatch;
  shared-expert chains, combine passes and ye-transposes are WOVEN between
  expert weight-chunk consumption to keep PE continuously busy (p-state) while
  DMA (the roofline, ~70MB of expert weights) never stalls.
"""

import numpy as np
import ml_dtypes

T, H, E, K, I = 512, 2048, 32, 8, 1408
NG, TKG = 8, 4
RSF = 2.5
C = 2 * T * K // E          # 256 per-expert capacity
NCORES = 8
P = 128
ISH = 2 * I // NCORES       # 352 shared-intermediate slice per core
HT = H // P                 # 16
TT = T // P                 # 4
ITL = I // P                # 11
GS = E // NG                # 4

bf16 = ml_dtypes.bfloat16


# ---------------------------------------------------------------------------
# Host routing (exact fp32 mirror of reference._route)
# ---------------------------------------------------------------------------
def _host_route(x, gate_w, bias):
    logits = x.astype(np.float32) @ gate_w.astype(np.float32)
    scores = (1.0 / (1.0 + np.exp(-logits.astype(np.float32)))).astype(np.float32)
    sb = scores + bias[None, :].astype(np.float32)
    g = sb.reshape(T, NG, GS)
    gs = np.sort(g, axis=-1)
    grp = gs[..., -1] + gs[..., -2]
    gidx = np.argsort(-grp, axis=-1, kind="stable")[:, :TKG]
    gmask = np.zeros((T, NG), bool)
    gmask[np.arange(T)[:, None], gidx] = True
    masked = np.where(gmask[:, :, None], g, -np.inf).reshape(T, E)
    topk = np.argsort(-masked, axis=-1, kind="stable")[:, :K]
    w = np.take_along_axis(scores, topk, axis=1)
    w = w / w.sum(-1, keepdims=True)
    return (w * RSF).astype(np.float32), topk.astype(np.int64)


def _plan(topk):
    loads = np.bincount(topk.reshape(-1), minlength=E)
    order = np.argsort(-loads, kind="stable")
    groups = [[int(order[j * NCORES + c]) for j in range(4)]
              for c in range(NCORES)]
    slot_caps = []
    for j in range(4):
        mx = max(min(int(loads[order[j * NCORES + c]]), C)
                 for c in range(NCORES))
        slot_caps.append(max(8, int(np.ceil(mx / 8) * 8)))
    return groups, slot_caps


def _build_dispatch(weights, topk, groups, slot_caps):
    """Per-core D [T, DCOLS] and WcT [P, NCB, T] (both fp32, cast later)."""
    offs = np.cumsum([0] + slot_caps)
    DCOLS = int(offs[-1])
    NCB = (DCOLS + P - 1) // P
    flat_e = topk.reshape(-1)
    tok = np.repeat(np.arange(T), K)
    wf = weights.reshape(-1)
    Ds, WcTs = [], []
    for core in range(NCORES):
        D = np.zeros((T, DCOLS), np.float32)
        Wc = np.zeros((T, NCB * P), np.float32)
        for j, e in enumerate(groups[core]):
            pos = np.flatnonzero(flat_e == e)[:C]
            r = np.arange(len(pos))
            D[tok[pos], offs[j] + r] = 1.0
            Wc[tok[pos], offs[j] + r] = wf[pos]
        WcT = np.ascontiguousarray(
            Wc.reshape(T, NCB, P).transpose(2, 1, 0))   # [P, NCB, T]
        Ds.append(D)
        WcTs.append(WcT)
    return Ds, WcTs, offs, DCOLS, NCB


# ---------------------------------------------------------------------------
# Host weight packing
# ---------------------------------------------------------------------------
def _pack_w13(w):
    """w [H, 2I] -> [P, ITL, HT*2P]: chunk i holds (g_i | u_i) per h-tile."""
    out = np.empty((P, ITL, HT * 2 * P), bf16)
    for i in range(ITL):
        for k in range(HT):
            blk = np.empty((P, 2 * P), np.float32)
            blk[:, :P] = w[k * P:(k + 1) * P, i * P:(i + 1) * P]
            blk[:, P:] = w[k * P:(k + 1) * P, I + i * P:I + (i + 1) * P]
            out[:, i, k * 2 * P:(k + 1) * 2 * P] = blk.astype(bf16)
    return out


def _pack_w2(w):
    """w [I, H] -> [P, 8, ITL*2P]: chunk hc holds h-tiles (2hc, 2hc+1)."""
    out = np.empty((P, 8, ITL * 2 * P), bf16)
    for hc in range(8):
        for ki in range(ITL):
            blk = np.empty((P, 2 * P), np.float32)
            blk[:, :P] = w[ki * P:(ki + 1) * P, (2 * hc) * P:(2 * hc + 1) * P]
            blk[:, P:] = w[ki * P:(ki + 1) * P, (2 * hc + 1) * P:(2 * hc + 2) * P]
            out[:, hc, ki * 2 * P:(ki + 1) * 2 * P] = blk.astype(bf16)
    return out


def _pack_wgu(sgu, core):
    """[H, 2*2816] -> [P, HT, 704] cols [g0|u0|g1|u1|g2(96)|u2(96)]."""
    gsl = sgu[:, core * ISH:(core + 1) * ISH]
    usl = sgu[:, 2 * I + core * ISH:2 * I + (core + 1) * ISH]
    segs = [(0, 128), (128, 256), (256, 352)]
    out = np.zeros((P, HT, 704), np.float32)
    for k in range(HT):
        col = 0
        for (a, b) in segs:
            w = b - a
            out[:, k, col:col + w] = gsl[k * P:(k + 1) * P, a:b]
            col += w
            out[:, k, col:col + w] = usl[k * P:(k + 1) * P, a:b]
            col += w
    return out.astype(bf16)


def _pack_wdn(sdn, core):
    out = np.zeros((P, 3, H), np.float32)
    sl = sdn[core * ISH:(core + 1) * ISH, :]
    for s in range(3):
        r = min(P, ISH - s * P)
        out[:r, s, :] = sl[s * P:s * P + r, :]
    return out.astype(bf16)


# ---------------------------------------------------------------------------
# Device program
# ---------------------------------------------------------------------------
def _build_nc(slot_caps, single_core=False):
    import concourse.mybir as mybir
    import concourse.tile as tile
    from concourse import bacc
    from contextlib import ExitStack
    from collections import deque

    f32 = mybir.dt.float32
    b16 = mybir.dt.bfloat16
    Alu = mybir.AluOpType
    Act = mybir.ActivationFunctionType

    offs = np.cumsum([0] + list(slot_caps)).tolist()
    DCOLS = offs[-1]
    NCB = (DCOLS + P - 1) // P
    CAPMAX = max(slot_caps)
    cw0 = ((DCOLS // 2) + 7) // 8 * 8
    cw1 = DCOLS - cw0
    YRING = 3

    nc = bacc.Bacc("TRN2", target_bir_lowering=False, debug=False,
                   num_devices=1 if single_core else NCORES)

    x_d = nc.dram_tensor("x_bf", [T, H], b16, kind="ExternalInput")
    d_d = nc.dram_tensor("disp", [T, DCOLS], b16, kind="ExternalInput")
    wct_d = nc.dram_tensor("wct", [P, NCB, T], b16, kind="ExternalInput")
    w13_d = nc.dram_tensor("w13p", [4, P, ITL * HT * 2 * P], b16,
                           kind="ExternalInput")
    w2_d = nc.dram_tensor("w2p", [4, P, 8 * ITL * 2 * P], b16,
                          kind="ExternalInput")
    wgu_d = nc.dram_tensor("wgup", [P, HT, 704], b16, kind="ExternalInput")
    wdn_d = nc.dram_tensor("wdnp", [P, 3, H], b16, kind="ExternalInput")
    id16_d = nc.dram_tensor("id_b16", [P, P], b16, kind="ExternalInput")
    out_d = nc.dram_tensor("out_slice",
                           [T, H] if single_core else [T // NCORES, H], b16,
                           kind="ExternalOutput")
    if not single_core:
        partial_d = nc.dram_tensor("partial", [T, H], b16, kind="Internal")
        rs_d = nc.dram_tensor("rs_out", [T // NCORES, H], b16, kind="Internal")

    xr = x_d.ap().rearrange("(tt p) h -> p tt h", p=P)
    dr = d_d.ap().rearrange("(tt p) c -> p tt c", p=P)

    # shared-gu packed col segments: (colofs, width) pairs g/u interleaved
    SSEG = [(0, 128), (256, 128), (512, 96)]       # gate col starts in wgu_p

    with tile.TileContext(nc) as tc, ExitStack() as ctx:
        pc = ctx.enter_context(tc.tile_pool(name="persist", bufs=1))
        w13sp = ctx.enter_context(tc.tile_pool(name="w13s", bufs=4))
        w2sp = ctx.enter_context(tc.tile_pool(name="w2s", bufs=3))
        ap_ = ctx.enter_context(tc.tile_pool(name="acts", bufs=2))
        tp_ = ctx.enter_context(tc.tile_pool(name="tmps", bufs=2))
        psS = ctx.enter_context(tc.tile_pool(name="psumS", bufs=2, space="PSUM"))
        psG = ctx.enter_context(tc.tile_pool(name="psumG", bufs=2, space="PSUM"))
        psU = ctx.enter_context(tc.tile_pool(name="psumU", bufs=2, space="PSUM"))
        psSG = ctx.enter_context(tc.tile_pool(name="psumSG", bufs=1, space="PSUM"))

        # ---- input DMAs in arrival order (SP queue) ----
        id16_sb = pc.tile([P, P], b16, tag="id16")
        nc.sync.dma_start(id16_sb[:], id16_d.ap())
        x_bf = pc.tile([P, TT, H], b16, tag="xb")
        nc.sync.dma_start(x_bf[:], xr)
        d_sb = pc.tile([P, TT, DCOLS], b16, tag="D")
        nc.sync.dma_start(d_sb[:], dr)
        wgu_sb = pc.tile([P, HT, 704], b16, tag="wgu")
        nc.sync.dma_start(wgu_sb[:], wgu_d.ap())
        wdn_sb = pc.tile([P, 3, H], b16, tag="wdn")
        nc.sync.dma_start(wdn_sb[:], wdn_d.ap())
        wct_sb = pc.tile([P, NCB, T], b16, tag="wct")
        nc.sync.dma_start(wct_sb[:], wct_d.ap())

        # ---- weight stream: consumption-order chunk list with lookahead ----
        stream = []
        for j in range(4):
            stream += [("w13", j, i) for i in range(ITL)]
            stream += [("w2", j, hc) for hc in range(8)]
        LOOKAHEAD = 5
        tiles = {}

        def issue(n):
            if n >= len(stream):
                return
            kind, j, i = stream[n]
            if kind == "w13":
                tl = w13sp.tile([P, HT * 2 * P], b16, tag="w13c", name="w13c")
                nc.sync.dma_start(
                    tl[:], w13_d.ap()[j][:, i * HT * 2 * P:(i + 1) * HT * 2 * P])
            else:
                tl = w2sp.tile([P, ITL * 2 * P], b16, tag="w2c", name="w2c")
                nc.sync.dma_start(
                    tl[:], w2_d.ap()[j][:, i * ITL * 2 * P:(i + 1) * ITL * 2 * P])
            tiles[n] = tl

        for n in range(LOOKAHEAD):
            issue(n)
        consumed = [0]

        def take():
            n = consumed[0]
            tl = tiles.pop(n)
            consumed[0] = n + 1
            issue(n + LOOKAHEAD)
            return tl

        # ---- persistent activations ----
        xT = pc.tile([P, HT, T], b16, tag="xT")
        xeT = pc.tile([P, HT, DCOLS], b16, tag="xeT")
        actShT = pc.tile([P, 3, T], b16, tag="actShT")
        peS = pc.tile([P, TT, H], b16, tag="peS")
        ye = pc.tile([P, YRING, H], b16, tag="ye")
        yeT = pc.tile([P, HT, NCB * P], b16, tag="yeT")
        if NCB * P > DCOLS:
            nc.gpsimd.memset(yeT[:, :, DCOLS:], 0.0)

        cpi = [0]

        def cp(out, in_):
            if cpi[0] % 2 == 0:
                nc.vector.tensor_copy(out=out, in_=in_)
            else:
                nc.scalar.copy(out, in_)
            cpi[0] += 1

        # ---- head: x transposes -> xT ----
        for s in range(HT // 2):
            for tt in range(TT):
                pt = psS.tile([P, 2 * P], b16, tag="sm", name="pt_x")
                for hl in range(2):
                    nc.tensor.transpose(
                        pt[:, hl * P:(hl + 1) * P],
                        x_bf[:, tt, (2 * s + hl) * P:(2 * s + hl + 1) * P],
                        id16_sb[:])
                cp(xT[:, 2 * s:2 * s + 2, tt * P:(tt + 1) * P],
                   pt.rearrange("p (k c) -> p k c", k=2))

        # ---- head: dispatch -> xeT ----
        for ko in range(HT):
            pg = psG.tile([P, 512], f32, tag="g", name="pg_d")
            pu = psU.tile([P, 512], f32, tag="u", name="pu_d")
            for tt in range(TT):
                nc.tensor.matmul(pg[:, :cw0], x_bf[:, tt, ko * P:(ko + 1) * P],
                                 d_sb[:, tt, :cw0],
                                 start=(tt == 0), stop=(tt == TT - 1))
                nc.tensor.matmul(pu[:, :cw1], x_bf[:, tt, ko * P:(ko + 1) * P],
                                 d_sb[:, tt, cw0:DCOLS],
                                 start=(tt == 0), stop=(tt == TT - 1))
            cp(xeT[:, ko, :cw0], pg[:, :cw0])
            cp(xeT[:, ko, cw0:DCOLS], pu[:, :cw1])

        # ---- woven PE work: shared-expert chains + combine passes ----
        pending = deque()

        def weave(k=1):
            for _ in range(k):
                if pending:
                    pending.popleft()()

        def shared_pair(mb):
            gcol, w = SSEG[mb]
            hold = {}

            def piece_g():
                ps = psSG.tile([P, 512], f32, tag="sg", name="ps_g")
                for k in range(HT):
                    nc.tensor.matmul(ps[:w, :], wgu_sb[:, k, gcol:gcol + w],
                                     xT[:, k, :],
                                     start=(k == 0), stop=(k == HT - 1))
                tmp = tp_.tile([P, 512], b16, tag="sgtmp", name="sgtmp", bufs=1)
                nc.scalar.activation(tmp[:w, :], ps[:w, :], Act.Silu)
                hold["tmp"] = tmp

            def piece_u():
                ps = psSG.tile([P, 512], f32, tag="su", name="ps_u")
                for k in range(HT):
                    nc.tensor.matmul(ps[:w, :],
                                     wgu_sb[:, k, gcol + w:gcol + 2 * w],
                                     xT[:, k, :],
                                     start=(k == 0), stop=(k == HT - 1))
                nc.vector.tensor_tensor(actShT[:w, mb, :], hold["tmp"][:w, :],
                                        ps[:w, :], Alu.mult)

            pending.append(piece_g)
            pending.append(piece_u)

        pass_first = [True]

        def combine_pass(cbs, with_shared, last):
            # one piece per (tt, hq): po = sum_cb WcT_cb^T @ ye_cb (+ shared)
            if not cbs and not with_shared:
                return
            for tt in range(TT):
                for hq in range(4):
                    def piece(tt=tt, hq=hq, cbs=tuple(cbs),
                              with_shared=with_shared, last=last,
                              first=pass_first[0]):
                        pool = psG if (tt + hq) % 2 == 0 else psU
                        tag = "g" if (tt + hq) % 2 == 0 else "u"
                        po = pool.tile([P, 512], f32, tag=tag, name="po")
                        nmm = (3 if with_shared else 0) + len(cbs)
                        q = 0
                        if with_shared:
                            for s in range(3):
                                r = min(P, ISH - s * P)
                                nc.tensor.matmul(
                                    po[:], actShT[:r, s, tt * P:(tt + 1) * P],
                                    wdn_sb[:r, s, hq * 512:(hq + 1) * 512],
                                    start=(q == 0), stop=(q == nmm - 1))
                                q += 1
                        for cb in cbs:
                            nc.tensor.matmul(
                                po[:], wct_sb[:, cb, tt * P:(tt + 1) * P],
                                ye[:, cb % YRING, hq * 512:(hq + 1) * 512],
                                start=(q == 0), stop=(q == nmm - 1))
                            q += 1
                        dst = peS[:, tt, hq * 512:(hq + 1) * 512]
                        if first:
                            nc.vector.tensor_copy(out=dst, in_=po[:])
                        elif not last:
                            nc.vector.tensor_tensor(dst, dst, po[:], Alu.add)
                        else:
                            stg = tp_.tile([P, 512], b16, tag="stg", name="stg")
                            nc.vector.tensor_tensor(stg[:], dst, po[:], Alu.add)
                            if single_core:
                                dd = out_d.ap()[tt * P:(tt + 1) * P,
                                                hq * 512:(hq + 1) * 512]
                            else:
                                dd = partial_d.ap()[tt * P:(tt + 1) * P,
                                                    hq * 512:(hq + 1) * 512]
                            nc.gpsimd.dma_start(dd, stg[:])
                    pending.append(piece)
            pass_first[0] = False

        # queue the shared gate/up chains first (needed before any pass that
        # includes the shared-down contribution)
        for mb in range(3):
            shared_pair(mb)

        # ---- expert pipeline ----
        cb_done = [0]

        def expert(j):
            cap = slot_caps[j]
            off = offs[j]
            act = ap_.tile([P, ITL, CAPMAX], b16, tag="act", name="act")
            for i in range(ITL):
                wg = take().rearrange("p (k f) -> p k f", f=2 * P)
                pg = psG.tile([P, 512], f32, tag="g", name="pg_w13")
                pu = psU.tile([P, 512], f32, tag="u", name="pu_w13")
                for k in range(HT):
                    nc.tensor.matmul(pg[:, :cap], wg[:, k, :P],
                                     xeT[:, k, off:off + cap],
                                     start=(k == 0), stop=(k == HT - 1))
                    nc.tensor.matmul(pu[:, :cap], wg[:, k, P:2 * P],
                                     xeT[:, k, off:off + cap],
                                     start=(k == 0), stop=(k == HT - 1))
                tmp = tp_.tile([P, 512], b16, tag="silu", name="silu")
                nc.scalar.activation(tmp[:, :cap], pg[:, :cap], Act.Silu)
                nc.vector.tensor_tensor(act[:, i, :cap], tmp[:, :cap],
                                        pu[:, :cap], Alu.mult)
                weave(1)
            for hc in range(8):
                w2c = take().rearrange("p (k f) -> p k f", f=2 * P)
                for m in range(2):
                    pool = psG if m == 0 else psU
                    py = pool.tile([P, 512], f32, tag="g" if m == 0 else "u",
                                   name="py")
                    for ki in range(ITL):
                        nc.tensor.matmul(py[:, :cap], w2c[:, ki, m * P:(m + 1) * P],
                                         act[:, ki, :cap],
                                         start=(ki == 0), stop=(ki == ITL - 1))
                    cp(yeT[:, 2 * hc + m, off:off + cap], py[:, :cap])
                weave(1)
            # transpose completed global c-blocks: yeT [h, c] -> ye [c, h]
            hi = (offs[j + 1] // P) if j < 3 else NCB
            cbs = list(range(cb_done[0], hi))
            cb_done[0] = hi
            for cb in cbs:
                for s in range(HT // 2):
                    pt = psS.tile([P, 2 * P], b16, tag="sm", name="pt_ye")
                    for hl in range(2):
                        nc.tensor.transpose(
                            pt[:, hl * P:(hl + 1) * P],
                            yeT[:, 2 * s + hl, cb * P:(cb + 1) * P],
                            id16_sb[:])
                    cp(ye[:, cb % YRING, 2 * s * P:(2 * s + 2) * P],
                       pt.rearrange("p (k c) -> p k c", k=2))
                    if s % 4 == 3:
                        weave(1)
            # schedule combine pass for tiles completed by this expert
            combine_pass(cbs, with_shared=(j == 1), last=(j == 3))

        for j in range(4):
            expert(j)
        while pending:
            pending.popleft()()

        if not single_core:
            nc.gpsimd.collective_compute(
                "ReduceScatter", Alu.add,
                replica_groups=[list(range(NCORES))],
                ins=[partial_d.ap().opt()],
                outs=[rs_d.ap().opt()],
            )
            nc.sync.dma_start(out_d.ap(), rs_d.ap())

    nc.compile()
    return nc


_NC_CACHE = {}


def _prepare(hidden_states, gate_w, bias):
    x = np.ascontiguousarray(np.asarray(hidden_states, np.float32))
    weights, topk = _host_route(x, np.ascontiguousarray(np.asarray(gate_w, np.float32)),
                                np.asarray(bias, np.float32))
    groups, slot_caps = _plan(topk)
    return x, weights, topk, groups, slot_caps


def kernel(hidden_states, residual, gate_w, bias, w13, w2, shared_gate_up,
           shared_down):
    from concourse.bass_utils import run_bass_kernel_spmd

    x, weights, topk, groups, slot_caps = _prepare(hidden_states, gate_w, bias)
    w13 = np.asarray(w13, np.float32)
    w2 = np.asarray(w2, np.float32)
    sgu = np.asarray(shared_gate_up, np.float32)
    sdn = np.asarray(shared_down, np.float32)

    Ds, WcTs, offs, DCOLS, NCB = _build_dispatch(weights, topk, groups,
                                                 slot_caps)

    key = tuple(slot_caps)
    if key not in _NC_CACHE:
        _NC_CACHE[key] = _build_nc(slot_caps)
    nc = _NC_CACHE[key]

    id16 = np.eye(P, dtype=np.float32).astype(bf16)
    x_bf = x.astype(bf16)
    in_maps = []
    for core in range(NCORES):
        in_maps.append({
            "x_bf": x_bf,
            "disp": Ds[core].astype(bf16),
            "wct": WcTs[core].astype(bf16),
            "w13p": np.stack([_pack_w13(w13[e]).reshape(P, -1)
                              for e in groups[core]]),
            "w2p": np.stack([_pack_w2(w2[e]).reshape(P, -1)
                             for e in groups[core]]),
            "wgup": _pack_wgu(sgu, core),
            "wdnp": _pack_wdn(sdn, core),
            "id_b16": id16,
        })
    res = run_bass_kernel_spmd(nc, in_maps, core_ids=list(range(NCORES)))
    out = np.concatenate([np.asarray(res.results[c]["out_slice"])
                          for c in range(NCORES)], axis=0)
    return out.astype(np.float32)
